# revision 19
# baseline (speedup 1.0000x reference)
"""CodeGen attention block (B=1, S=2048, E=2048, H=16, D=128, rot=64) on 8 TRN2
NeuronCores.

Sharding: tensor-parallel over heads (2 heads/core). Each core computes its
heads' q/k in transposed [d, s] layout (partial rotary applied via a host-side
even/odd channel permutation folded into the qkv weights), v in [s, d] layout,
causal softmax attention on-chip (scores transposed [k, q]), AllGathers the
per-core attention output O^T per q chunk, and computes a 256-column slice of
the output projection. Host assembles and transposes.

v3 optimizations:
 - softmax denominators via bf16 ptsum accumulation on vector + one gpsimd
   partition_all_reduce per (head, chunk) (removes 80 full-cost PE matmuls).
 - causal diagonal trimming on QK / exp / PV / ptsum.
 - startup: hidden loaded in 512-col quarters for q chunks 0/1 (critical)
   and a deferred 1024-col half for chunks 2/3; warm-up matmuls hold the
   HAM clock gate open; a dummy AllGather warms the collective engine.
 - background interleave: qkv(j+1) / outproj matmuls are emitted between
   attention tiles so the PE fills softmax-paced bubbles.
 - chunk-3 AllGather split per head so the final output projection overlaps
   the last collective.
"""

import numpy as np

H, D, ROT, MP = 16, 128, 64, 4
S, E = 2048, 2048
NCORES = 8
P = 128
NQ = 4            # 512-wide q chunks
NKT = S // P      # 16 k tiles
NEC = E // P      # 16 contraction chunks
SCALE = float(1.0 / np.sqrt(np.float64(D)))

_CACHE = {}


# ----------------------------------------------------------------------------
# host-side input prep
# ----------------------------------------------------------------------------

def _head_rows(h):
    g, j = h // 4, h % 4
    base = g * (3 * 512)
    q = np.arange(base + j * 128, base + (j + 1) * 128)
    v = np.arange(base + 512 + j * 128, base + 512 + (j + 1) * 128)
    k = np.arange(base + 1024 + j * 128, base + 1024 + (j + 1) * 128)
    return q, v, k


def _prep_core_weights(c, w_qkv, w_out):
    h0, h1 = 2 * c, 2 * c + 1
    top = np.arange(0, ROT, 2)
    bot = np.arange(1, ROT, 2)
    wq, wk, wv = {}, {}, {}
    for h in (h0, h1):
        qr, vr, kr = _head_rows(h)
        wq[h], wv[h], wk[h] = w_qkv[qr], w_qkv[vr], w_qkv[kr]
    G0 = np.concatenate([wq[h0][top], wq[h1][top], wk[h0][top], wk[h1][top]], 0)
    G1 = np.concatenate([wq[h0][bot], wq[h1][bot], wk[h0][bot], wk[h1][bot]], 0)
    G2 = np.concatenate([wq[h0][ROT:], wq[h1][ROT:]], 0)
    G3 = np.concatenate([wk[h0][ROT:], wk[h1][ROT:]], 0)
    wqkT = np.ascontiguousarray(
        np.concatenate([G0, G1, G2, G3], 0).T, dtype=np.float32)       # [E, 512]
    wvT = np.ascontiguousarray(
        np.concatenate([wv[h0], wv[h1]], 0).T, dtype=np.float32)       # [E, 256]
    wqkvT = np.ascontiguousarray(np.concatenate([wqkT, wvT], 1))       # [E, 768]
    woutT = np.ascontiguousarray(
        w_out[256 * c:256 * (c + 1), :].T, dtype=np.float32)           # [E, 256]
    woutT = np.ascontiguousarray(
        woutT.reshape(16, 128, 256).transpose(1, 0, 2)).reshape(128, 16 * 256)
    return wqkvT, woutT


def _cos_sin():
    inv_freq = 1.0 / (10000.0 ** (np.arange(0, ROT, 2, dtype=np.float32) / ROT))
    ang = np.arange(S, dtype=np.float32)[:, None] * inv_freq[None, :]
    cosb = np.cos(ang).T.astype(np.float32)        # [32, S]
    sinb = np.sin(ang).T.astype(np.float32)
    return (np.ascontiguousarray(np.tile(cosb, (4, 1))),
            np.ascontiguousarray(np.tile(sinb, (4, 1))))               # [128, S]


def _tri_mask():
    pp = np.arange(128)[:, None]
    cc = np.arange(128)[None, :]
    return (pp <= cc).astype(np.float32)


def _build_in_maps(hidden_states, w_qkv, w_out):
    import ml_dtypes
    bf16 = ml_dtypes.bfloat16
    hiddenT = np.ascontiguousarray(
        np.asarray(hidden_states, np.float32).reshape(S, E).T).astype(bf16)
    COS, SIN = _cos_sin()
    COS, SIN = COS.astype(bf16), SIN.astype(bf16)
    tri = _tri_mask().astype(bf16)
    in_maps = []
    for c in range(NCORES):
        wqkvT, woutT = _prep_core_weights(
            c, np.asarray(w_qkv, np.float32), np.asarray(w_out, np.float32))
        in_maps.append({
            "hiddenT": hiddenT,
            "wqkvT": wqkvT.astype(bf16),
            "woutT": woutT.astype(bf16),
            "cosT": COS,
            "sinT": SIN,
            "trimask": tri,
        })
    return in_maps


# ----------------------------------------------------------------------------
# device program
# ----------------------------------------------------------------------------

def _kernel_body(tc, outT, hiddenT, wqkvT, woutT, cosT, sinT, trimaskT):
    import concourse.mybir as mybir
    from contextlib import ExitStack

    nc = tc.nc
    f32 = mybir.dt.float32
    bt = mybir.dt.bfloat16

    with ExitStack() as ctx:
        const = ctx.enter_context(tc.tile_pool(name="const", bufs=1))
        tri_sb = const.tile([P, P], bt, name="tri_sb")
        warm_sb = const.tile([P, 512], bt, name="warm_sb")
        warm_st = const.tile([P, P], bt, name="warm_st")
        ones_col = const.tile([P, 1], bt, name="ones_col")
        ones_row = const.tile([1, P], bt, name="ones_row")
        wo_sb = const.tile([P, NEC, 256], bt, name="wo_sb")

        dram = ctx.enter_context(tc.tile_pool(name="dram", bufs=1, space="DRAM"))
        agw_in = dram.tile([P, 64], bt, name="agw_in", tag="agw_in")
        agw_out = dram.tile([NCORES * P, 64], bt, name="agw_out", tag="agw_out",
                            addr_space="Shared")
        ag_in = [dram.tile([2 * P, 512], bt, name=f"ag_in{j}", tag=f"ag_in{j}")
                 for j in range(NQ - 1)]
        ag_out = [dram.tile([E, 512], bt, name=f"ag_out{j}", tag=f"ag_out{j}",
                            addr_space="Shared") for j in range(NQ - 1)]
        ag3_in = [dram.tile([P, 512], bt, name=f"ag3_in{h}", tag=f"ag3_in{h}")
                  for h in range(2)]
        ag3_out = [dram.tile([NCORES * P, 512], bt, name=f"ag3_out{h}",
                             tag=f"ag3_out{h}", addr_space="Shared")
                   for h in range(2)]

        with tc.tile_pool(name="ph1c", bufs=1) as ph1c_pool, \
             tc.tile_pool(name="chunk", bufs=1) as ck_pool, \
             tc.tile_pool(name="hidq", bufs=NEC) as hidq_pool, \
             tc.tile_pool(name="wts", bufs=NEC) as wts_pool, \
             tc.tile_pool(name="g01c", bufs=2) as g01_pool, \
             tc.tile_pool(name="rtmp", bufs=1) as rtmp_pool, \
             tc.tile_pool(name="pt", bufs=6) as pt_pool, \
             tc.tile_pool(name="ptsums", bufs=2) as ps_pool, \
             tc.tile_pool(name="oout", bufs=2) as oo_pool, \
             tc.tile_pool(name="otin", bufs=2) as oi_pool, \
             tc.tile_pool(name="oprj", bufs=3) as op_pool, \
             tc.tile_pool(name="gps", bufs=2, space="PSUM") as gps_pool, \
             tc.tile_pool(name="scps", bufs=2, space="PSUM") as sc_pool, \
             tc.tile_pool(name="otps", bufs=2, space="PSUM") as ot_pool, \
             tc.tile_pool(name="opps", bufs=2, space="PSUM") as pp_pool:

            # ---- warm-up: hold the PE clock gate open + warm the CC engine --
            nc.vector.memset(warm_st[:], 0.25)
            nc.vector.memset(warm_sb[:], 0.25)
            nc.vector.memset(ones_col[:], 1.0)
            nc.vector.memset(ones_row[:], 1.0)
            nc.gpsimd.dma_start(agw_in[:], warm_sb[:, 0:64])
            nc.gpsimd.collective_compute(
                "AllGather", mybir.AluOpType.bypass,
                replica_groups=[list(range(NCORES))],
                ins=[agw_in.opt()], outs=[agw_out.opt()])

            def warm_mms(n):
                for _ in range(n):
                    wp = pp_pool.tile([P, 512], f32, name="warm", tag="pps")
                    nc.tensor.matmul(wp[:], warm_st[:], warm_sb[:],
                                     start=True, stop=True)

            warm_mms(6)

            # ---- DMA issue order: strict priority, critical 6MB spread
            # round-robin over all three DMA-capable queues ----
            qs3 = (nc.gpsimd, nc.sync, nc.scalar)
            wqkv_sb = []
            hid_q0 = [None] * NEC
            hid_rest = [None] * NEC

            for e in range(NEC):
                wt = wts_pool.tile([P, 768], bt, name=f"wqkv_{e}", tag="wqkv",
                                   bufs=NEC)
                qs3[e % 3].dma_start(wt[:], wqkvT[e * P:(e + 1) * P, :])
                wqkv_sb.append(wt)
                ht = hidq_pool.tile([P, 512], bt, name=f"hidq_{e}",
                                    tag="hidq", bufs=NEC)
                qs3[(e + 1) % 3].dma_start(ht[:],
                                           hiddenT[e * P:(e + 1) * P, 0:512])
                hid_q0[e] = ht
            cos_sb = ph1c_pool.tile([P, S], bt, name="cos_sb", tag="cos_sb")
            sin_sb = ph1c_pool.tile([P, S], bt, name="sin_sb", tag="sin_sb")
            nc.gpsimd.dma_start(cos_sb[:], cosT)
            nc.sync.dma_start(sin_sb[:], sinT)
            nc.scalar.dma_start(tri_sb[:], trimaskT)
            # q chunks 1-3 as one wide load per e-chunk (3KB lines, fewer
            # descriptor-generation stalls); then w_out. Everything is in the
            # prologue: the first collective cannot begin until the DMA burst
            # drains, so nothing is gained by deferring and late loads only
            # inflate AllGather wire time.
            for e in range(NEC):
                ht = hidq_pool.tile([P, 3 * 512], bt, name=f"hidr_{e}",
                                    tag="hidr", bufs=NEC)
                qs3[(e + 2) % 3].dma_start(ht[:],
                                           hiddenT[e * P:(e + 1) * P, 512:2048])
                hid_rest[e] = ht
            nc.gpsimd.dma_start(wo_sb[:], woutT.rearrange(
                "p (o e) -> p o e", o=NEC))

            # per-chunk activations: q/k transposed [d, 512]; v [k-in-tile, d]
            qc = [[ck_pool.tile([P, 512], bt, name=f"qc{h}_{j}", tag=f"qc{h}_{j}")
                   for j in range(NQ)] for h in range(2)]
            kc = [[ck_pool.tile([P, 512], bt, name=f"kc{h}_{j}", tag=f"kc{h}_{j}")
                   for j in range(NQ)] for h in range(2)]
            vc = [[ck_pool.tile([P, 512], bt, name=f"vc{h}_{j}", tag=f"vc{h}_{j}")
                   for j in range(NQ)] for h in range(2)]

            oti_sb = [None] * NQ
            oti3_sb = [None, None]

            def hidsl(e, j, lo=0, width=512):
                if j == 0:
                    return hid_q0[e][:, lo:lo + width]
                off = (j - 1) * 512 + lo
                return hid_rest[e][:, off:off + width]

            def qkv_thunks(j, split_v=False):
                """Emission thunks for qkv chunk j, in dependency order.
                split_v=True returns (thunks, v-tail thunks) so the v tail
                can run in the background of attention chunk j."""
                th = []
                g01 = [None, None]

                def mk_gchain(g):
                    gp = gps_pool.tile([P, 512], f32, name=f"gps{j}_{g}",
                                       tag="gps")
                    for e in range(NEC):
                        if j == 0 and g == 0:
                            # qkv(0) g0 is DMA-paced: keep the HAM clock gate
                            # open with filler matmuls in the arrival gaps
                            th.append(lambda: warm_mms(2))
                        th.append(lambda g=g, e=e, gp=gp: nc.tensor.matmul(
                            gp[:], wqkv_sb[e][:, g * P:(g + 1) * P],
                            hidsl(e, j), start=(e == 0), stop=(e == NEC - 1)))
                    if g in (0, 1):
                        def drain(g=g, gp=gp):
                            gc = g01_pool.tile([P, 512], bt, name=f"g01_{j}_{g}",
                                               tag=f"g01_{g}")
                            nc.scalar.copy(gc[:], gp[:])
                            g01[g] = gc
                        th.append(drain)
                    else:
                        dst = qc if g == 2 else kc
                        th.append(lambda dst=dst, gp=gp: nc.vector.tensor_copy(
                            dst[0][j][64:128, :], gp[0:64, :]))
                        th.append(lambda dst=dst, gp=gp: nc.vector.tensor_copy(
                            dst[1][j][64:128, :], gp[64:128, :]))

                for g in (0, 1, 2, 3):
                    mk_gchain(g)

                def mk_vchain(st, dst):
                    vp = gps_pool.tile([P, 256], f32, name=f"vps{j}_{st}",
                                       tag="gps")
                    for e in range(NEC):
                        dst.append(lambda st=st, e=e, vp=vp: nc.tensor.matmul(
                            vp[:], hidsl(e, j, st * P, P),
                            wqkv_sb[e][:, 512:768], start=(e == 0),
                            stop=(e == NEC - 1)))
                    dst.append(lambda st=st, vp=vp: nc.vector.tensor_copy(
                        vc[0][j][:, st * P:(st + 1) * P], vp[:, 0:P]))
                    dst.append(lambda st=st, vp=vp: nc.vector.tensor_copy(
                        vc[1][j][:, st * P:(st + 1) * P], vp[:, P:2 * P]))

                def rope():
                    qs = slice(512 * j, 512 * (j + 1))
                    t0 = rtmp_pool.tile([P, 512], bt, name=f"t0_{j}", tag="t0")
                    t1 = rtmp_pool.tile([P, 512], bt, name=f"t1_{j}", tag="t1")
                    ta = rtmp_pool.tile([P, 512], bt, name=f"ta_{j}", tag="ta")
                    tb = rtmp_pool.tile([P, 512], bt, name=f"tb_{j}", tag="tb")
                    nc.vector.tensor_mul(t0[:], g01[0][:], cos_sb[:, qs])
                    nc.vector.tensor_mul(t1[:], g01[1][:], sin_sb[:, qs])
                    nc.vector.tensor_sub(ta[:], t0[:], t1[:])      # tops
                    nc.vector.tensor_mul(t0[:], g01[1][:], cos_sb[:, qs])
                    nc.vector.tensor_mul(t1[:], g01[0][:], sin_sb[:, qs])
                    nc.vector.tensor_add(tb[:], t0[:], t1[:])      # bottoms
                    for pc, dst in enumerate((qc[0], qc[1], kc[0], kc[1])):
                        ps = slice(32 * pc, 32 * (pc + 1))
                        nc.vector.tensor_copy(dst[j][0:32, :], ta[ps, :])
                        nc.vector.tensor_copy(dst[j][32:64, :], tb[ps, :])
                th.append(rope)
                v_tail = []
                mk_vchain(0, th)
                for st in (1, 2, 3):
                    mk_vchain(st, v_tail if split_v else th)
                return (th, v_tail) if split_v else th

            def run_thunks(th):
                for t in th:
                    t()

            def attn_head(j, hi, bg):
                """Attention for (chunk j, head hi); returns otn tile.
                bg: iterator of background thunks, a few run per tile."""
                nk = 4 * j + 4
                otp = ot_pool.tile([P, 512], f32, name=f"otp{hi}_{j}", tag="otp")
                ptsum = ps_pool.tile([P, 512], bt, name=f"pts{hi}_{j}",
                                     tag="pts")
                for i in range(nk):
                    mm = i - 4 * j
                    lo = 128 * mm if mm > 0 else 0
                    scp = sc_pool.tile([P, 512], f32, name=f"scp{hi}{j}{i}",
                                       tag="scp")
                    nc.tensor.matmul(
                        scp[:, lo:512],
                        kc[hi][i // 4][:, (i % 4) * P:(i % 4 + 1) * P],
                        qc[hi][j][:, lo:512], start=True, stop=True)
                    pt = pt_pool.tile([P, 512], bt, name=f"pt{hi}{j}{i}",
                                      tag="pt")
                    nc.scalar.activation(
                        pt[:, lo:512], scp[:, lo:512],
                        mybir.ActivationFunctionType.Exp, scale=SCALE)
                    if mm >= 0:
                        nc.vector.tensor_mul(
                            pt[:, lo:lo + P], pt[:, lo:lo + P], tri_sb[:])
                    if i == 0:
                        nc.vector.tensor_copy(ptsum[:], pt[:])
                    else:
                        nc.vector.tensor_add(ptsum[:, lo:512],
                                             ptsum[:, lo:512], pt[:, lo:512])
                    nc.tensor.matmul(
                        otp[:, lo:512],
                        vc[hi][i // 4][:, (i % 4) * P:(i % 4 + 1) * P],
                        pt[:, lo:512], start=(i == 0), stop=(i == nk - 1))
                    for _ in range(3):
                        t = next(bg, None)
                        if t is None:
                            break
                        t()
                otn = oo_pool.tile([P, 512], bt, name=f"otn{hi}_{j}", tag="otn")
                # low-latency denominator: partition-reduce and broadcast via
                # two tiny PE matmuls (keeps the AllGather data-ready chain
                # short and the gpsimd queue free for DMA/collective triggers)
                den_ps = sc_pool.tile([1, 512], f32, name=f"denp{hi}_{j}",
                                      tag="scp")
                nc.tensor.matmul(den_ps[:], ones_col[:], ptsum[:],
                                 start=True, stop=True)
                rden1f = oo_pool.tile([1, 512], f32, name=f"rden1f_{hi}",
                                      tag="rden1f")
                nc.vector.reciprocal_approx_fast(rden1f[:], den_ps[:])
                rden1 = oo_pool.tile([1, 512], bt, name=f"rden1_{hi}",
                                     tag="rden1")
                nc.vector.tensor_copy(rden1[:], rden1f[:])
                dbc_ps = sc_pool.tile([P, 512], f32, name=f"dbc{hi}_{j}",
                                      tag="scp")
                nc.tensor.matmul(dbc_ps[:], ones_row[:], rden1[:],
                                 start=True, stop=True)
                dbc_sb = oo_pool.tile([P, 512], f32, name=f"dbcs{hi}_{j}",
                                      tag="den")
                nc.scalar.copy(dbc_sb[:], dbc_ps[:])
                nc.vector.tensor_mul(otn[:], otp[:], dbc_sb[:])
                return otn

            def emit_attn(j, bg):
                for hi in range(2):
                    otn = attn_head(j, hi, bg)
                    nc.sync.dma_start(ag_in[j][hi * P:(hi + 1) * P, :], otn[:])
                nc.gpsimd.collective_compute(
                    "AllGather", mybir.AluOpType.bypass,
                    replica_groups=[list(range(NCORES))],
                    ins=[ag_in[j].opt()], outs=[ag_out[j].opt()])
                oti = oi_pool.tile([P, NEC, 512], bt, name=f"oti{j}", tag="oti")
                nc.sync.dma_start(
                    oti[:], ag_out[j].rearrange("(o p) s -> p o s", p=P))
                oti_sb[j] = oti

            def emit_attn3(bg):
                # chunk 3: per-head AllGather so the tail overlaps out-proj
                for hi in range(2):
                    otn = attn_head(3, hi, bg)
                    nc.sync.dma_start(ag3_in[hi][:, :], otn[:])
                    nc.gpsimd.collective_compute(
                        "AllGather", mybir.AluOpType.bypass,
                        replica_groups=[list(range(NCORES))],
                        ins=[ag3_in[hi].opt()], outs=[ag3_out[hi].opt()])
                    oti = oi_pool.tile([P, NCORES, 512], bt, name=f"oti3_{hi}",
                                       tag="oti")
                    nc.sync.dma_start(
                        oti[:], ag3_out[hi].rearrange("(o p) s -> p o s", p=P))
                    oti3_sb[hi] = oti

            def oproj_thunks(j):
                qs = slice(512 * j, 512 * (j + 1))
                th = []
                pps = [pp_pool.tile([P, 512], f32, name=f"pps{j}_{b}", tag="pps")
                       for b in range(2)]
                for fc in range(NEC):
                    for b in range(2):
                        th.append(lambda fc=fc, b=b: nc.tensor.matmul(
                            pps[b][:], wo_sb[:, fc, b * P:(b + 1) * P],
                            oti_sb[j][:, fc, :], start=(fc == 0),
                            stop=(fc == NEC - 1)))
                for b in range(2):
                    def drain(b=b):
                        ob = op_pool.tile([P, 512], f32, name=f"ob{j}_{b}",
                                          tag="ob")
                        nc.vector.tensor_copy(ob[:], pps[b][:])
                        nc.sync.dma_start(outT[b * P:(b + 1) * P, qs], ob[:])
                    th.append(drain)
                return th

            def emit_oproj3():
                qs = slice(512 * 3, 512 * 4)
                pps = [pp_pool.tile([P, 512], f32, name=f"pps3_{b}", tag="pps")
                       for b in range(2)]
                for hi in range(2):
                    for c in range(NCORES):
                        for b in range(2):
                            nc.tensor.matmul(
                                pps[b][:], wo_sb[:, 2 * c + hi,
                                                 b * P:(b + 1) * P],
                                oti3_sb[hi][:, c, :],
                                start=(hi == 0 and c == 0),
                                stop=(hi == 1 and c == NCORES - 1))
                for b in range(2):
                    ob = op_pool.tile([P, 512], f32, name=f"ob3_{b}", tag="ob")
                    nc.vector.tensor_copy(ob[:], pps[b][:])
                    nc.sync.dma_start(outT[b * P:(b + 1) * P, qs], ob[:])

            # ---- main schedule ----
            run_thunks(qkv_thunks(0))
            emit_attn(0, iter(()))
            run_thunks(qkv_thunks(1))

            bg = iter(qkv_thunks(2))
            emit_attn(1, bg)
            run_thunks(list(bg))            # rest of qkv(2)
            run_thunks(oproj_thunks(0))

            bg = iter(qkv_thunks(3))
            emit_attn(2, bg)
            run_thunks(list(bg))            # rest of qkv(3)
            run_thunks(oproj_thunks(1))

            bg = iter(oproj_thunks(2))
            emit_attn3(bg)
            run_thunks(list(bg))            # rest of outproj(2)
            emit_oproj3()


def _build_program():
    import concourse.bass as bass  # noqa: F401
    import concourse.mybir as mybir
    import concourse.tile as tile
    from concourse import bacc

    nc = bacc.Bacc("TRN2", target_bir_lowering=False, debug=False,
                   enable_asserts=False, num_devices=NCORES)
    f32 = mybir.dt.float32
    bt = mybir.dt.bfloat16
    hiddenT = nc.dram_tensor("hiddenT", [E, S], bt, kind="ExternalInput").ap()
    wqkvT = nc.dram_tensor("wqkvT", [E, 768], bt, kind="ExternalInput").ap()
    woutT = nc.dram_tensor("woutT", [P, NEC * 256], bt, kind="ExternalInput").ap()
    cosT = nc.dram_tensor("cosT", [P, S], bt, kind="ExternalInput").ap()
    sinT = nc.dram_tensor("sinT", [P, S], bt, kind="ExternalInput").ap()
    trimask = nc.dram_tensor("trimask", [P, P], bt, kind="ExternalInput").ap()
    outT = nc.dram_tensor("outT", [2 * P, S], f32, kind="ExternalOutput").ap()

    with tile.TileContext(nc) as tc:
        _kernel_body(tc, outT, hiddenT, wqkvT, woutT, cosT, sinT, trimask)
    nc.compile()
    return nc


def get_program():
    if "nc" not in _CACHE:
        _CACHE["nc"] = _build_program()
    return _CACHE["nc"]


def _install_ntff_shim():
    """Provide antenv.axon_hooks (missing in this image) so trace=True works."""
    import sys
    import types
    try:
        import antenv.axon_hooks  # noqa: F401
        return
    except ImportError:
        pass
    import antenv
    mod = types.ModuleType("antenv.axon_hooks")
    mod._hook = None

    def set_axon_ntff_profile_hook(h):
        mod._hook = h

    def get_axon_ntff_profile_hook():
        return mod._hook

    mod.set_axon_ntff_profile_hook = set_axon_ntff_profile_hook
    mod.get_axon_ntff_profile_hook = get_axon_ntff_profile_hook
    sys.modules["antenv.axon_hooks"] = mod
    antenv.axon_hooks = mod
    try:
        from trn_agent_boot.trn_boot import _ntff_profile_via_ctypes
        hook = _ntff_profile_via_ctypes("/opt/axon/libaxon_pjrt.so")
        if hook is not None:
            mod._hook = hook
    except Exception:
        pass


def run(inputs, trace=False):
    """Run on the 8 NeuronCores; returns (out [1,S,E], BassKernelResults)."""
    from concourse import bass_utils

    if trace:
        _install_ntff_shim()
    nc = get_program()
    in_maps = _build_in_maps(inputs["hidden_states"], inputs["w_qkv"],
                             inputs["w_out"])
    res = bass_utils.run_bass_kernel_spmd(
        nc, in_maps, core_ids=list(range(NCORES)), trace=trace)
    outT = np.concatenate([res.results[c]["outT"] for c in range(NCORES)],
                          axis=0)  # [E, S]
    out = np.ascontiguousarray(outT.T).reshape(1, S, E).astype(np.float32)
    return out, res


def kernel(hidden_states, w_qkv, w_out):
    out, _ = run({"hidden_states": hidden_states, "w_qkv": w_qkv,
                  "w_out": w_out})
    return out
